# revision 25
# baseline (speedup 1.0000x reference)
"""Trainium2 Bass kernel for nn_DiffusionModule (B=2, L=768, C=256, H=8, NB=4).

Sharding: sequence-parallel over the 768 residues across 8 NeuronCores
(96 query rows + the matching 96-row slab of `pair` per core). Params are
replicated. Per transformer block one bf16 AllGather of the adaLN'd
activations provides full-length K/V inputs.

The pair-bias projection for all 4 blocks is computed in one pass over the
pair shard (cast to bf16 during the DMA), with the result held in SBUF in a
transposed [j-partition, (block,head)] layout using a mod-6 interleaved
j-permutation (j = 6*p + kappa) that falls out of contiguous loads +
128x128 PE transposes. Attention runs entirely in that permuted j order
(valid: softmax + AV contract over j), with transposed scores [j, i],
no max-subtraction (logits are O(1) for this module), and the softmax
denominator obtained from a ones-column in the V tile during the AV matmul.
"""

import math
import os
import sys

for _p in ("/opt/trn_rl_repo", "/root/.axon_site/_ro/trn_rl_repo"):
    if os.path.isdir(_p) and _p not in sys.path:
        sys.path.insert(0, _p)

import numpy as np
import ml_dtypes

import concourse.bass as bass
import concourse.bacc as bacc
import concourse.tile as tile
from concourse import mybir
from concourse.bass_utils import run_bass_kernel_spmd

F32 = mybir.dt.float32
BF16 = mybir.dt.bfloat16
AF = mybir.ActivationFunctionType

B, L, C, CS, CZ, H, NB = 2, 768, 256, 256, 64, 8, 4
HD = C // H            # 32
NCORES = 8
LLOC = L // NCORES     # 96
NK = 6                 # j-chunks: j = 6*p + kappa, p in [0,128)
CH = NB * H            # 32 pair-bias channels (all blocks x heads)
IB = 8                 # i-rows per pair staging DMA
SCALE = 1.0 / math.sqrt(HD)

_CACHED = {}
_LAST = {"exec_time_ns": None, "results": None}


def _install_ntff_hook():
    """Shim antenv.axon_hooks (absent in this image) so trace=True works."""
    try:
        import antenv.axon_hooks  # noqa: F401
        return
    except ImportError:
        pass
    import types
    import antenv
    hooks = types.ModuleType("antenv.axon_hooks")
    box = {"h": None}
    hooks.set_axon_ntff_profile_hook = lambda h: box.__setitem__("h", h)
    hooks.get_axon_ntff_profile_hook = lambda: box["h"]
    antenv.axon_hooks = hooks
    sys.modules["antenv.axon_hooks"] = hooks
    try:
        if "/root/.axon_site" not in sys.path:
            sys.path.append("/root/.axon_site")
        from trn_agent_boot import trn_boot
        so = "/opt/axon/libaxon_pjrt.so"
        if os.path.exists(so):
            hooks.set_axon_ntff_profile_hook(trn_boot._ntff_profile_via_ctypes(so))
    except Exception:
        pass


_install_ntff_hook()


def _ap(src, offset, dims):
    """Raw access pattern on the tensor behind AP/TensorHandle `src`.

    `offset` is relative to `src`'s own offset (elements)."""
    if isinstance(src, bass.AP):
        t, base = src.tensor, src.offset
    else:
        a = src[:]
        t, base = a.tensor, a.offset
    return bass.AP(tensor=t, offset=base + offset, ap=[list(d) for d in dims])


def build_nc():
    nc = bacc.Bacc("TRN2", target_bir_lowering=False, debug=False, num_devices=NCORES)

    def din(name, shape, dtype=F32):
        return nc.dram_tensor(name, list(shape), dtype, kind="ExternalInput")

    pair_loc = din("pair_loc", [B, LLOC, L, CZ])
    rots_loc = din("rots_loc", [B, LLOC, 9])
    trans_loc = din("trans_loc", [B, LLOC, 3])
    single_loc = din("single_loc", [B, LLOC, CS])
    t_in = din("t", [B])
    frame_w = din("frame_w", [12, C]); frame_b = din("frame_b", [1, C])
    single_w = din("single_w", [CS, C]); single_b = din("single_b", [1, C])
    tw1 = din("tw1", [C, 4 * C]); tb1 = din("tb1", [1, 4 * C])
    tw2 = din("tw2", [4 * C, C]); tb2 = din("tb2", [1, C])
    out_w = din("out_w", [C, 6]); out_b = din("out_b", [1, 6])
    ag1 = din("ag1", [NB, C]); abeta1 = din("abeta1", [NB, C])
    apw1 = din("apw1", [NB, C, 2 * C]); apb1 = din("apb1", [NB, 2 * C])
    ag2 = din("ag2", [NB, C]); abeta2 = din("abeta2", [NB, C])
    apw2 = din("apw2", [NB, C, 2 * C]); apb2 = din("apb2", [NB, 2 * C])
    wq = din("wq", [NB, C, C]); wk = din("wk", [NB, C, C])
    wv = din("wv", [NB, C, C]); wo = din("wo", [NB, C, C])
    wob = din("wob", [NB, C]); pw = din("pw", [NB, CZ, H])
    fw1 = din("fw1", [NB, C, 4 * C]); fb1 = din("fb1", [NB, 4 * C])
    fw2 = din("fw2", [NB, 4 * C, C]); fb2 = din("fb2", [NB, C])
    freqs = din("freqs", [1, C // 2])
    eye_f = din("eye_f", [128, 128])
    eye_b = din("eye_b", [128, 128], BF16)
    out_d = nc.dram_tensor("out", [B, LLOC, 12], F32, kind="ExternalOutput")

    with tile.TileContext(nc) as tc:
        import contextlib
        ctx = contextlib.ExitStack()
        with ctx:
            P = ctx.enter_context(tc.tile_pool(name="persist", bufs=1))
            work = ctx.enter_context(tc.tile_pool(name="work", bufs=2))
            bcpool = ctx.enter_context(tc.tile_pool(name="bcast", bufs=2))
            ps_t = ctx.enter_context(tc.tile_pool(name="ps_t", bufs=2, space="PSUM"))
            ps_s = ctx.enter_context(tc.tile_pool(name="ps_s", bufs=2, space="PSUM"))
            ps_a = ctx.enter_context(tc.tile_pool(name="ps_a", bufs=2, space="PSUM"))
            ps_m = ctx.enter_context(tc.tile_pool(name="ps_m", bufs=2, space="PSUM"))
            dram = ctx.enter_context(tc.tile_pool(name="dram", bufs=2, space="DRAM"))
            dramP = ctx.enter_context(tc.tile_pool(name="dramP", bufs=1, space="DRAM"))
            hpool = ctx.enter_context(tc.tile_pool(name="hpool", bufs=4))

            def psum(pool, shape, dtype=F32, tag=""):
                tg = tag or {id(ps_t): "t", id(ps_s): "s", id(ps_a): "a", id(ps_m): "m"}[id(pool)]
                return pool.tile(shape, dtype, tag=tg, name=f"ps{tg}_{nc.next_id()}")

            # ---------- constants ----------
            eyef_sb = P.tile([128, 128], F32)
            nc.sync.dma_start(out=eyef_sb, in_=eye_f[:])
            eyeb_sb = P.tile([128, 128], BF16)
            nc.sync.dma_start(out=eyeb_sb, in_=eye_b[:])
            ones_f = P.tile([1, 128], F32); nc.vector.memset(ones_f, 1.0)
            ones_b = P.tile([1, 128], BF16); nc.vector.memset(ones_b, 1.0)
            eps_ln = P.tile([128, 1], F32); nc.vector.memset(eps_ln, 1e-5)
            halfpi = P.tile([128, 1], F32); nc.vector.memset(halfpi, math.pi / 2)
            eps8 = P.tile([128, 1], F32); nc.vector.memset(eps8, 1e-8)
            one_c = P.tile([128, 1], F32); nc.vector.memset(one_c, 1.0)

            setup_ctx = contextlib.ExitStack()
            setup = setup_ctx.enter_context(tc.tile_pool(name="setup", bufs=1))

            # ---------- resident weights (bf16 via SWDGE cast-DMA) ----------
            def cast_w(src, blk, kc, n, name):
                tl = P.tile([128, kc, n], BF16, name=name)
                nc.gpsimd.dma_start(
                    out=tl, in_=_ap(src, blk * kc * 128 * n, [[n, 128], [128 * n, kc], [1, n]]))
                return tl

            wq_sb = [cast_w(wq, i, 2, C, f"wq{i}") for i in range(NB)]
            wk_sb = [cast_w(wk, i, 2, C, f"wk{i}") for i in range(NB)]
            wv_sb = [cast_w(wv, i, 2, C, f"wv{i}") for i in range(NB)]
            wo_sb = [cast_w(wo, i, 2, C, f"wo{i}") for i in range(NB)]
            fw1_sb = [cast_w(fw1, i, 2, 4 * C, f"fw1_{i}") for i in range(NB)]
            fw2_sb = [cast_w(fw2, i, 8, C, f"fw2_{i}") for i in range(NB)]

            pw_bd = P.tile([128, 2 * CH], BF16)
            nc.vector.memset(pw_bd, 0.0)
            for s in range(2):
                nc.gpsimd.dma_start(
                    out=pw_bd[s * CZ:(s + 1) * CZ, s * CH:s * CH + CH],
                    in_=_ap(pw, 0, [[H, CZ], [CZ * H, NB], [1, H]]))

            wob_sb = P.tile([1, NB * C], BF16)
            nc.gpsimd.dma_start(out=wob_sb, in_=_ap(wob, 0, [[NB * C, 1], [1, NB * C]]))
            fb2_sb = P.tile([1, NB * C], BF16)
            nc.gpsimd.dma_start(out=fb2_sb, in_=_ap(fb2, 0, [[NB * C, 1], [1, NB * C]]))

            # fb1 columns: [128, 8(hid-chunk), NB]
            fb1_sb = P.tile([128, 8, NB], F32)
            for k in range(8):
                fb1_nat = setup.tile([NB, 128], F32, tag="fb1n")
                nc.sync.dma_start(out=fb1_nat, in_=_ap(
                    fb1, k * 128, [[4 * C, NB], [1, 128]]))
                tps = psum(ps_t, [128, NB], F32)
                nc.tensor.transpose(tps, fb1_nat, eyef_sb[0:NB, 0:NB])
                nc.any.tensor_copy(out=fb1_sb[:, k, :], in_=tps)

            outw_sb = P.tile([128, 2, 6], F32)
            nc.sync.dma_start(out=outw_sb, in_=_ap(out_w, 0, [[6, 128], [768, 2], [1, 6]]))
            outb_sb = P.tile([1, 6], F32)
            nc.sync.dma_start(out=outb_sb, in_=out_b[:])

            frame_w_sb = setup.tile([12, C], F32)
            nc.sync.dma_start(out=frame_w_sb, in_=frame_w[:])
            single_w_sb = setup.tile([128, 2, C], F32)
            nc.sync.dma_start(out=single_w_sb, in_=_ap(single_w, 0, [[C, 128], [128 * C, 2], [1, C]]))
            cb_f = setup.tile([1, C], F32)
            cb_s = work.tile([1, C], F32)
            nc.sync.dma_start(out=cb_f, in_=frame_b[:])
            nc.sync.dma_start(out=cb_s, in_=single_b[:])
            nc.vector.tensor_add(out=cb_f, in0=cb_f, in1=cb_s)  # frame_b + single_b

            # ---------- h init ----------
            rots_sb, trans_sb, h_sb = [], [], []
            for b in range(B):
                rt = P.tile([LLOC, 9], F32, name=f"rots{b}")
                nc.sync.dma_start(out=rt, in_=rots_loc[b])
                tr = P.tile([LLOC, 3], F32, name=f"trans{b}")
                nc.sync.dma_start(out=tr, in_=trans_loc[b])
                rots_sb.append(rt); trans_sb.append(tr)

                ff = setup.tile([LLOC, 12], F32)
                nc.vector.tensor_copy(out=ff[:, 0:9], in_=rt)
                nc.vector.tensor_copy(out=ff[:, 9:12], in_=tr)
                ffT_ps = psum(ps_t, [12, LLOC], F32)
                nc.tensor.transpose(ffT_ps, ff, eyef_sb[0:LLOC, 0:LLOC])
                ffT = setup.tile([12, LLOC], F32)
                nc.any.tensor_copy(out=ffT, in_=ffT_ps)

                sg = setup.tile([LLOC, CS], F32)
                nc.sync.dma_start(out=sg, in_=single_loc[b])
                sgT = setup.tile([128, 2, LLOC], F32)
                for cc in range(2):
                    sps = psum(ps_t, [128, LLOC], F32)
                    nc.tensor.transpose(sps, sg[:, cc * 128:(cc + 1) * 128], eyef_sb[0:LLOC, 0:LLOC])
                    nc.any.tensor_copy(out=sgT[:, cc, :], in_=sps)

                hps = psum(ps_m, [LLOC, C], F32)
                nc.tensor.matmul(hps, ffT, frame_w_sb, start=True, stop=False)
                for cc in range(2):
                    nc.tensor.matmul(hps, sgT[:, cc, :], single_w_sb[:, cc, :],
                                     start=False, stop=False)
                nc.tensor.matmul(hps, ones_f[:, 0:LLOC], cb_f, start=False, stop=True)
                ht = hpool.tile([LLOC, C], F32, tag=f"h{b}", name=f"h_{b}")
                nc.vector.tensor_copy(out=ht, in_=hps)
                h_sb.append(ht)

            # ---------- time embedding -> adaLN row vectors ----------
            tb1_sb = setup.tile([1, 4 * C], F32)
            nc.sync.dma_start(out=tb1_sb, in_=tb1[:])
            tb2_sb = setup.tile([1, C], F32)
            nc.sync.dma_start(out=tb2_sb, in_=tb2[:])

            tsb = setup.tile([B, 1], F32)
            nc.sync.dma_start(out=tsb, in_=_ap(t_in, 0, [[1, B], [1, 1]]))
            fr2 = setup.tile([B, C // 2], F32)
            nc.sync.dma_start(out=fr2, in_=_ap(freqs, 0, [[0, B], [1, C // 2]]))
            targ = setup.tile([B, C // 2], F32)
            nc.vector.tensor_scalar_mul(out=targ, in0=fr2, scalar1=tsb)
            temb = setup.tile([B, C], F32)
            nc.scalar.activation(out=temb[:, 0:C // 2], in_=targ, func=AF.Sin,
                                 bias=halfpi[0:B], scale=1.0)
            nc.scalar.activation(out=temb[:, C // 2:C], in_=targ, func=AF.Sin)

            tembT = setup.tile([128, 2, B], F32)
            for cc in range(2):
                tps = psum(ps_t, [128, B], F32)
                nc.tensor.transpose(tps, temb[:, cc * 128:(cc + 1) * 128], eyef_sb[0:B, 0:B])
                nc.any.tensor_copy(out=tembT[:, cc, :], in_=tps)

            gT = setup.tile([128, 8, B], F32)
            for half in range(2):
                hd_ps = psum(ps_m, [B, 512], F32)
                for cc in range(2):
                    tw1_s = setup.tile([128, 512], F32, tag="tw1s")
                    nc.sync.dma_start(out=tw1_s, in_=_ap(
                        tw1, cc * 128 * 1024 + half * 512, [[1024, 128], [1, 512]]))
                    nc.tensor.matmul(hd_ps, tembT[:, cc, :], tw1_s,
                                     start=(cc == 0), stop=False)
                nc.tensor.matmul(hd_ps, ones_f[:, 0:B], tb1_sb[:, half * 512:(half + 1) * 512],
                                 start=False, stop=True)
                gmlp_h = setup.tile([B, 512], F32, tag="gmlph")
                nc.scalar.activation(out=gmlp_h, in_=hd_ps, func=AF.Gelu)
                for k4 in range(4):
                    tps = psum(ps_t, [128, B], F32)
                    nc.tensor.transpose(tps, gmlp_h[:, k4 * 128:(k4 + 1) * 128],
                                        eyef_sb[0:B, 0:B])
                    nc.any.tensor_copy(out=gT[:, half * 4 + k4, :], in_=tps)
            tc_ps = psum(ps_m, [B, C], F32)
            for k in range(8):
                tw2_s = setup.tile([128, C], F32, tag="tw2s")
                nc.sync.dma_start(out=tw2_s, in_=_ap(
                    tw2, k * 128 * C, [[C, 128], [1, C]]))
                nc.tensor.matmul(tc_ps, gT[:, k, :], tw2_s, start=(k == 0), stop=False)
            nc.tensor.matmul(tc_ps, ones_f[:, 0:B], tb2_sb, start=False, stop=True)
            tcond = setup.tile([B, C], F32)
            nc.vector.tensor_copy(out=tcond, in_=tc_ps)
            tcT = setup.tile([128, 2, B], F32)
            for cc in range(2):
                tps = psum(ps_t, [128, B], F32)
                nc.tensor.transpose(tps, tcond[:, cc * 128:(cc + 1) * 128], eyef_sb[0:B, 0:B])
                nc.any.tensor_copy(out=tcT[:, cc, :], in_=tps)

            # adaLN (m, s) row vectors for all (blk, which, b), staged in DRAM
            # so they can be partition-broadcast-loaded at block time.
            mrow_d = dramP.tile([NB * 2 * B, C], F32)
            srow_d = dramP.tile([NB * 2 * B, C], F32)
            apw_l = [apw1, apw2]; apb_l = [apb1, apb2]
            ag_l = [ag1, ag2]; ab_l = [abeta1, abeta2]
            for blk in range(NB):
                for wch in range(2):
                    apw_sb = setup.tile([128, 2, 2 * C], F32, tag="apw")
                    nc.sync.dma_start(out=apw_sb, in_=_ap(
                        apw_l[wch], blk * C * 2 * C, [[2 * C, 128], [128 * 2 * C, 2], [1, 2 * C]]))
                    apb_sb = setup.tile([1, 2 * C], F32, tag="apb")
                    nc.sync.dma_start(out=apb_sb, in_=_ap(apb_l[wch], blk * 2 * C, [[0, 1], [1, 2 * C]]))
                    ss_ps = psum(ps_m, [B, 2 * C], F32)
                    for cc in range(2):
                        nc.tensor.matmul(ss_ps, tcT[:, cc, :], apw_sb[:, cc, :],
                                         start=(cc == 0), stop=False)
                    nc.tensor.matmul(ss_ps, ones_f[:, 0:B], apb_sb, start=False, stop=True)
                    ag_bc = setup.tile([B, C], F32, tag="agbc")
                    nc.sync.dma_start(out=ag_bc, in_=_ap(ag_l[wch], blk * C, [[0, B], [1, C]]))
                    ab_bc = setup.tile([B, C], F32, tag="abbc")
                    nc.sync.dma_start(out=ab_bc, in_=_ap(ab_l[wch], blk * C, [[0, B], [1, C]]))
                    onep = setup.tile([B, C], F32, tag="onep")
                    nc.vector.tensor_scalar_add(out=onep, in0=ss_ps[:, 0:C], scalar1=1.0)
                    mr = setup.tile([B, C], F32, tag="mr")
                    nc.vector.tensor_mul(out=mr, in0=onep, in1=ag_bc)
                    sr = setup.tile([B, C], F32, tag="sr")
                    nc.vector.tensor_mul(out=sr, in0=onep, in1=ab_bc)
                    nc.vector.tensor_add(out=sr, in0=sr, in1=ss_ps[:, C:2 * C])
                    row = (blk * 2 + wch) * B
                    nc.sync.dma_start(out=mrow_d[row:row + B, :], in_=mr)
                    nc.sync.dma_start(out=srow_d[row:row + B, :], in_=sr)

            setup_ctx.close()

            # ---------- pair bias for all blocks ----------
            slabp = ctx.enter_context(tc.tile_pool(name="slab", bufs=3))
            ptp = ctx.enter_context(tc.tile_pool(name="pairT", bufs=4))
            escp = ctx.enter_context(tc.tile_pool(name="esc", bufs=8))
            bias_sb = P.tile([128, B * LLOC * NK * CH], BF16)  # [128, 36864]
            with nc.named_scope("pairproj"):
                for b in range(B):
                    for i0 in range(0, LLOC, IB):
                        slab = slabp.tile([128, IB, 384], BF16, tag="slab")
                        nc.gpsimd.dma_start(out=slab, in_=_ap(
                            pair_loc, (b * LLOC + i0) * L * CZ,
                            [[384, 128], [L * CZ, IB], [1, 384]]))
                        for ii in range(IB):
                            i = i0 + ii
                            for t3 in range(3):
                                pt_ps = psum(ps_t, [128, 128], BF16)
                                nc.tensor.transpose(
                                    pt_ps, slab[:, ii, t3 * 128:(t3 + 1) * 128], eyeb_sb)
                                ptsb = ptp.tile([128, 128], BF16, tag="pt")
                                nc.any.tensor_copy(out=ptsb, in_=pt_ps)
                                bps = psum(ps_s, [128, 2 * CH], F32)
                                nc.tensor.matmul(bps, ptsb, pw_bd, start=True, stop=True)
                                off = ((b * LLOC + i) * NK + 2 * t3) * CH
                                nc.any.tensor_copy(out=bias_sb[:, off:off + 2 * CH], in_=bps)

            # ---------- transformer blocks ----------
            kT_sb = [P.tile([128, 2, L], BF16, name=f"kT{b}") for b in range(B)]
            vaug = [P.tile([128, NK, 33 * H], BF16, name=f"vaug{b}") for b in range(B)]
            for b in range(B):
                nc.vector.memset(vaug[b], 1.0)
            qT_sb = [P.tile([128, 2, LLOC], BF16, name=f"qT{b}") for b in range(B)]
            oT_sb = [P.tile([128, 2, LLOC], BF16, name=f"oT{b}") for b in range(B)]
            hhT_sb = [P.tile([128, 2, LLOC], BF16, name=f"hhT{b}") for b in range(B)]
            hhTf_sb = [P.tile([128, 2, L], BF16, name=f"hhTf{b}") for b in range(B)]
            h2T_sb = [P.tile([128, 2, LLOC], BF16, name=f"h2T{b}") for b in range(B)]

            def adaln(blk, wch, b, src):
                """adaLN of src [LLOC, C] f32 -> bf16 tile [LLOC, C]."""
                stats = work.tile([LLOC, 6], F32, tag="bnst")
                nc.vector.bn_stats(out=stats, in_=src)
                mv = work.tile([LLOC, 2], F32, tag="bnmv")
                nc.vector.bn_aggr(out=mv, in_=stats)
                nc.scalar.activation(out=mv[:, 1:2], in_=mv[:, 1:2], func=AF.Sqrt,
                                     bias=eps_ln[0:LLOC], scale=1.0)
                nc.vector.reciprocal(out=mv[:, 1:2], in_=mv[:, 1:2])
                xh = work.tile([LLOC, C], F32, tag="xh")
                nc.vector.tensor_scalar(out=xh, in0=src, scalar1=mv[:, 0:1],
                                        scalar2=mv[:, 1:2],
                                        op0=mybir.AluOpType.subtract,
                                        op1=mybir.AluOpType.mult)
                off = ((blk * 2 + wch) * B + b) * C
                m_bc = bcpool.tile([LLOC, C], F32, tag="mbc")
                nc.sync.dma_start(out=m_bc, in_=_ap(mrow_d, off, [[0, LLOC], [1, C]]))
                s_bc = bcpool.tile([LLOC, C], F32, tag="sbc")
                nc.sync.dma_start(out=s_bc, in_=_ap(srow_d, off, [[0, LLOC], [1, C]]))
                nc.vector.tensor_mul(out=xh, in0=xh, in1=m_bc)
                ob = work.tile([LLOC, C], BF16, tag="adaout")
                nc.vector.tensor_add(out=ob, in0=xh, in1=s_bc)
                return ob

            def transpose_to(dst, src_bf):
                """src_bf [LLOC, C] bf16 -> dst [128, 2, LLOC] bf16 (PE transpose)."""
                for cc in range(2):
                    tps = psum(ps_t, [128, LLOC], BF16)
                    nc.tensor.transpose(tps, src_bf[:, cc * 128:(cc + 1) * 128],
                                        eyeb_sb[0:LLOC, 0:LLOC])
                    nc.any.tensor_copy(out=dst[:, cc, :], in_=tps)

            bias_r = bias_sb.rearrange("p (bb ii kk cc) -> p bb ii kk cc",
                                       bb=B, ii=LLOC, kk=NK, cc=CH)

            for blk in range(NB):
                with nc.named_scope(f"blk{blk}"):
                    cc_in = dram.tile([B, 128, 2, LLOC], BF16, tag="ccin")
                    for b in range(B):
                        hh = adaln(blk, 0, b, h_sb[b])
                        transpose_to(hhT_sb[b], hh)
                        nc.sync.dma_start(out=cc_in[b], in_=hhT_sb[b])
                        # local q while the collective runs
                        for dc in range(2):
                            qps = psum(ps_m, [128, LLOC], F32)
                            for cc in range(2):
                                nc.tensor.matmul(
                                    qps, wq_sb[blk][:, cc, dc * 128:(dc + 1) * 128],
                                    hhT_sb[b][:, cc, :], start=(cc == 0), stop=(cc == 1))
                            nc.scalar.activation(out=qT_sb[b][:, dc, :], in_=qps,
                                                 func=AF.Copy, scale=SCALE)

                    cc_out = dram.tile([NCORES, B, 128, 2, LLOC], BF16, tag="ccout")
                    nc.gpsimd.collective_compute(
                        "AllGather", mybir.AluOpType.bypass,
                        replica_groups=[list(range(NCORES))],
                        ins=[cc_in.opt()], outs=[cc_out.opt()])

                    for b in range(B):
                        for cc in range(2):
                            nc.sync.dma_start(out=hhTf_sb[b][:, cc, :], in_=_ap(
                                cc_out, b * (128 * 2 * LLOC) + cc * LLOC,
                                [[2 * LLOC, 128], [B * 128 * 2 * LLOC, NCORES], [1, LLOC]]))
                        # K^T: [d, j] tiles
                        for dc in range(2):
                            for half, n0, nn in ((0, 0, 512), (1, 512, 256)):
                                kps = psum(ps_m, [128, nn], F32, tag="m")
                                for cc in range(2):
                                    nc.tensor.matmul(
                                        kps, wk_sb[blk][:, cc, dc * 128:(dc + 1) * 128],
                                        hhTf_sb[b][:, cc, n0:n0 + nn],
                                        start=(cc == 0), stop=(cc == 1))
                                nc.any.tensor_copy(out=kT_sb[b][:, dc, n0:n0 + nn], in_=kps)
                        # V (permuted j order): [j, d] tiles + ones column
                        for kap in range(NK):
                            vps = psum(ps_m, [128, C], F32)
                            for cc in range(2):
                                lh = hhTf_sb[b][:, cc, :].rearrange(
                                    "p (n six) -> p six n", six=NK)[:, kap, :]
                                nc.tensor.matmul(vps, lh, wv_sb[blk][:, cc, :],
                                                 start=(cc == 0), stop=(cc == 1))
                            nc.any.tensor_copy(
                                out=vaug[b].rearrange("p k (hh tt) -> p k hh tt",
                                                      hh=H)[:, kap, :, 0:HD],
                                in_=vps.rearrange("p (hh dd) -> p hh dd", hh=H))

                        # attention per head (scores transposed [j, i]; AV output
                        # in natural [i, d] so normalization is per-partition)
                        o_nat = work.tile([LLOC, C], BF16, tag="onat")
                        for h in range(H):
                            dc, r0 = h // 4, (h % 4) * HD
                            qTh = work.tile([128, LLOC], BF16, tag="qTh")
                            nc.vector.memset(qTh, 0.0)
                            nc.vector.tensor_copy(out=qTh[r0:r0 + HD, :],
                                                  in_=qT_sb[b][r0:r0 + HD, dc, :])
                            avps = psum(ps_a, [LLOC, 33], F32)
                            for kap in range(NK):
                                sps = psum(ps_s, [128, LLOC], F32)
                                lh = kT_sb[b][:, dc, :].rearrange(
                                    "p (n six) -> p six n", six=NK)[:, kap, :]
                                nc.tensor.matmul(sps, lh, qTh, start=True, stop=True)
                                badd = work.tile([128, LLOC], F32, tag="badd")
                                nc.vector.tensor_add(
                                    out=badd, in0=sps,
                                    in1=bias_r[:, b, :, kap, blk * H + h])
                                esc = escp.tile([128, LLOC], BF16, tag="esc")
                                nc.scalar.activation(out=esc, in_=badd, func=AF.Exp)
                                nc.tensor.matmul(
                                    avps, esc, vaug[b][:, kap, h * 33:(h + 1) * 33],
                                    start=(kap == 0), stop=(kap == NK - 1))
                            rcp = work.tile([LLOC, 1], F32, tag="rcp")
                            nc.vector.reciprocal(out=rcp, in_=avps[:, 32:33])
                            nc.vector.tensor_scalar_mul(
                                out=o_nat[:, h * HD:(h + 1) * HD],
                                in0=avps[:, 0:HD], scalar1=rcp)
                        transpose_to(oT_sb[b], o_nat)

                        # h += o @ wo + wob
                        ups = psum(ps_m, [LLOC, C], F32)
                        for cc in range(2):
                            nc.tensor.matmul(ups, oT_sb[b][:, cc, :], wo_sb[blk][:, cc, :],
                                             start=(cc == 0), stop=False)
                        nc.tensor.matmul(ups, ones_b[:, 0:LLOC], wob_sb[:, blk * C:(blk + 1) * C],
                                         start=False, stop=True)
                        hmid = hpool.tile([LLOC, C], F32, tag=f"h{b}", name=f"hmid{blk}_{b}")
                        nc.vector.tensor_add(out=hmid, in0=h_sb[b], in1=ups)

                        # FFN
                        h2 = adaln(blk, 1, b, hmid)
                        transpose_to(h2T_sb[b], h2)
                        gT = work.tile([128, 8, LLOC], BF16, tag="gT")
                        for mc in range(8):
                            gps = psum(ps_m, [128, LLOC], F32)
                            for cc in range(2):
                                nc.tensor.matmul(
                                    gps, fw1_sb[blk][:, cc, mc * 128:(mc + 1) * 128],
                                    h2T_sb[b][:, cc, :], start=(cc == 0), stop=(cc == 1))
                            nc.scalar.activation(out=gT[:, mc, :], in_=gps, func=AF.Gelu,
                                                 bias=fb1_sb[:, mc, blk:blk + 1], scale=1.0)
                        fps = psum(ps_m, [LLOC, C], F32)
                        for mc in range(8):
                            nc.tensor.matmul(fps, gT[:, mc, :], fw2_sb[blk][:, mc, :],
                                             start=(mc == 0), stop=False)
                        nc.tensor.matmul(fps, ones_b[:, 0:LLOC], fb2_sb[:, blk * C:(blk + 1) * C],
                                         start=False, stop=True)
                        hnew = hpool.tile([LLOC, C], F32, tag=f"h{b}", name=f"hnew{blk}_{b}")
                        nc.vector.tensor_add(out=hnew, in0=hmid, in1=fps)
                        h_sb[b] = hnew

            # ---------- output head: corr -> rodrigues -> compose ----------
            with nc.named_scope("outhead"):
                for b in range(B):
                    hT = work.tile([128, 2, LLOC], F32, tag="hT")
                    for cc in range(2):
                        tps = psum(ps_t, [128, LLOC], F32)
                        nc.tensor.transpose(tps, h_sb[b][:, cc * 128:(cc + 1) * 128],
                                            eyef_sb[0:LLOC, 0:LLOC])
                        nc.any.tensor_copy(out=hT[:, cc, :], in_=tps)
                    cps = psum(ps_m, [LLOC, 6], F32)
                    for cc in range(2):
                        nc.tensor.matmul(cps, hT[:, cc, :], outw_sb[:, cc, :],
                                         start=(cc == 0), stop=False)
                    nc.tensor.matmul(cps, ones_f[:, 0:LLOC], outb_sb, start=False, stop=True)
                    corr = work.tile([LLOC, 6], F32, tag="corr")
                    nc.vector.tensor_copy(out=corr, in_=cps)

                    v3 = corr[:, 0:3]
                    vv = work.tile([LLOC, 3], F32, tag="vv")
                    nc.vector.tensor_mul(out=vv, in0=v3, in1=v3)
                    n2 = work.tile([LLOC, 1], F32, tag="n2")
                    nc.vector.reduce_sum(out=n2, in_=vv, axis=mybir.AxisListType.X)
                    nrm = work.tile([LLOC, 1], F32, tag="nrm")
                    nc.scalar.activation(out=nrm, in_=n2, func=AF.Sqrt)
                    sinn = work.tile([LLOC, 1], F32, tag="sinn")
                    nc.scalar.activation(out=sinn, in_=nrm, func=AF.Sin)
                    cosn = work.tile([LLOC, 1], F32, tag="cosn")
                    nc.scalar.activation(out=cosn, in_=nrm, func=AF.Sin,
                                         bias=halfpi[0:LLOC], scale=1.0)
                    rn = work.tile([LLOC, 1], F32, tag="rn")
                    nc.vector.tensor_scalar_add(out=rn, in0=nrm, scalar1=1e-8)
                    nc.vector.reciprocal(out=rn, in_=rn)
                    ax = work.tile([LLOC, 3], F32, tag="ax")
                    nc.vector.tensor_scalar_mul(out=ax, in0=v3, scalar1=rn)
                    sa = work.tile([LLOC, 3], F32, tag="sa")
                    nc.vector.tensor_scalar_mul(out=sa, in0=ax, scalar1=sinn)
                    omc = work.tile([LLOC, 1], F32, tag="omc")
                    nc.vector.tensor_scalar(out=omc, in0=cosn, scalar1=-1.0,
                                            scalar2=1.0,
                                            op0=mybir.AluOpType.mult,
                                            op1=mybir.AluOpType.add)
                    R = work.tile([LLOC, 9], F32, tag="R")
                    for r in range(3):
                        nc.vector.tensor_scalar_mul(out=R[:, 3 * r:3 * r + 3], in0=ax,
                                                    scalar1=ax[:, r:r + 1])
                    nc.vector.tensor_scalar_mul(out=R, in0=R, scalar1=omc)
                    diag = _ap(R, 0, [list(R.ap[0]), [4, 3]])
                    nc.vector.tensor_scalar_add(out=diag, in0=diag, scalar1=cosn)
                    for col, src, sgn in ((1, 2, -1), (2, 1, +1), (3, 2, +1),
                                          (5, 0, -1), (6, 1, -1), (7, 0, +1)):
                        fn = nc.vector.tensor_add if sgn > 0 else nc.vector.tensor_sub
                        fn(out=R[:, col:col + 1], in0=R[:, col:col + 1],
                           in1=sa[:, src:src + 1])

                    res = work.tile([LLOC, 12], F32, tag="res")
                    tmp3 = work.tile([LLOC, 3], F32, tag="tmp3")
                    for r in range(3):
                        dst = res[:, 3 * r:3 * r + 3]
                        nc.vector.tensor_scalar_mul(out=dst, in0=R[:, 0:3],
                                                    scalar1=rots_sb[b][:, 3 * r:3 * r + 1])
                        for k in (1, 2):
                            nc.vector.tensor_scalar_mul(
                                out=tmp3, in0=R[:, 3 * k:3 * k + 3],
                                scalar1=rots_sb[b][:, 3 * r + k:3 * r + k + 1])
                            nc.vector.tensor_add(out=dst, in0=dst, in1=tmp3)
                    # new_trans = rots @ t_upd + trans
                    tup = corr[:, 3:6]
                    t1 = work.tile([LLOC, 3], F32, tag="t1")
                    t2 = work.tile([LLOC, 3], F32, tag="t2")
                    # rows of rots: res_t[r] = sum_k rots[3r+k]*tup[k]
                    rots_rk = rots_sb[b].rearrange("p (r k) -> p r k", k=3)
                    nc.vector.tensor_scalar_mul(out=t1, in0=rots_rk[:, :, 0],
                                                scalar1=tup[:, 0:1])
                    for k in (1, 2):
                        nc.vector.tensor_scalar_mul(out=t2, in0=rots_rk[:, :, k],
                                                    scalar1=tup[:, k:k + 1])
                        nc.vector.tensor_add(out=t1, in0=t1, in1=t2)
                    nc.vector.tensor_add(out=res[:, 9:12], in0=t1, in1=trans_sb[b])
                    nc.sync.dma_start(out=out_d[b], in_=res)

    nc.compile()
    return nc


def _inputs_to_maps(inputs):
    ins = {k: np.ascontiguousarray(np.asarray(v, dtype=np.float32)) for k, v in inputs.items()}
    half = C // 2
    freqs = np.exp(-math.log(10000.0) * np.arange(half, dtype=np.float32) / half)
    common = {
        "t": ins["t"],
        "frame_w": ins["frame_w"], "frame_b": ins["frame_b"].reshape(1, C),
        "single_w": ins["single_w"], "single_b": ins["single_b"].reshape(1, C),
        "tw1": ins["tw1"], "tb1": ins["tb1"].reshape(1, 4 * C),
        "tw2": ins["tw2"], "tb2": ins["tb2"].reshape(1, C),
        "out_w": ins["out_w"], "out_b": ins["out_b"].reshape(1, 6),
        "ag1": ins["ag1"], "abeta1": ins["abeta1"],
        "apw1": ins["apw1"], "apb1": ins["apb1"],
        "ag2": ins["ag2"], "abeta2": ins["abeta2"],
        "apw2": ins["apw2"], "apb2": ins["apb2"],
        "wq": ins["wq"], "wk": ins["wk"], "wv": ins["wv"], "wo": ins["wo"],
        "wob": ins["wob"], "pw": ins["pw"],
        "fw1": ins["fw1"], "fb1": ins["fb1"], "fw2": ins["fw2"], "fb2": ins["fb2"],
        "freqs": freqs.reshape(1, half),
        "eye_f": np.eye(128, dtype=np.float32),
        "eye_b": np.eye(128).astype(ml_dtypes.bfloat16),
    }
    maps = []
    rots9 = ins["rots"].reshape(B, L, 9)
    for c in range(NCORES):
        sl = slice(c * LLOC, (c + 1) * LLOC)
        m = dict(common)
        m["pair_loc"] = np.ascontiguousarray(ins["pair"][:, sl])
        m["rots_loc"] = np.ascontiguousarray(rots9[:, sl])
        m["trans_loc"] = np.ascontiguousarray(ins["trans"][:, sl])
        m["single_loc"] = np.ascontiguousarray(ins["single"][:, sl])
        maps.append(m)
    return maps


def kernel(**inputs):
    if "nc" not in _CACHED:
        _CACHED["nc"] = build_nc()
    nc = _CACHED["nc"]
    maps = _inputs_to_maps(inputs)
    res = run_bass_kernel_spmd(nc, maps, core_ids=list(range(NCORES)))
    _LAST["exec_time_ns"] = res.exec_time_ns
    _LAST["results"] = res
    out = np.concatenate([res.results[c]["out"] for c in range(NCORES)], axis=1)
    return out.astype(np.float32)


# revision 31
# speedup vs baseline: 1.2686x; 1.2686x over previous
"""Trainium2 Bass kernel for nn_DiffusionModule (B=2, L=768, C=256, H=8, NB=4).

Sharding: sequence-parallel over the 768 residues across 8 NeuronCores
(96 query rows + the matching 96-row slab of `pair` per core). Params are
replicated. Per transformer block one bf16 AllGather of the adaLN'd
activations provides full-length K/V inputs.

The pair-bias projection for all 4 blocks is computed in one pass over the
pair shard (cast to bf16 during the DMA), with the result held in SBUF in a
transposed [j-partition, (block,head)] layout using a mod-6 interleaved
j-permutation (j = 6*p + kappa) that falls out of contiguous loads +
128x128 PE transposes. Attention runs entirely in that permuted j order
(valid: softmax + AV contract over j), with transposed scores [j, i],
no max-subtraction (logits are O(1) for this module), and the softmax
denominator obtained from a ones-column in the V tile during the AV matmul.
"""

import math
import os
import sys

for _p in ("/opt/trn_rl_repo", "/root/.axon_site/_ro/trn_rl_repo"):
    if os.path.isdir(_p) and _p not in sys.path:
        sys.path.insert(0, _p)

import numpy as np
import ml_dtypes

import concourse.bass as bass
import concourse.bacc as bacc
import concourse.tile as tile
from concourse import mybir
from concourse.bass_utils import run_bass_kernel_spmd

F32 = mybir.dt.float32
BF16 = mybir.dt.bfloat16
AF = mybir.ActivationFunctionType

B, L, C, CS, CZ, H, NB = 2, 768, 256, 256, 64, 8, 4
HD = C // H            # 32
NCORES = 8
LLOC = L // NCORES     # 96
NK = 6                 # j-chunks: j = 6*p + kappa, p in [0,128)
CH = NB * H            # 32 pair-bias channels (all blocks x heads)
IB = 8                 # i-rows per pair staging DMA
SCALE = 1.0 / math.sqrt(HD)

_CACHED = {}
_LAST = {"exec_time_ns": None, "results": None}


def _install_ntff_hook():
    """Shim antenv.axon_hooks (absent in this image) so trace=True works."""
    try:
        import antenv.axon_hooks  # noqa: F401
        return
    except ImportError:
        pass
    import types
    import antenv
    hooks = types.ModuleType("antenv.axon_hooks")
    box = {"h": None}
    hooks.set_axon_ntff_profile_hook = lambda h: box.__setitem__("h", h)
    hooks.get_axon_ntff_profile_hook = lambda: box["h"]
    antenv.axon_hooks = hooks
    sys.modules["antenv.axon_hooks"] = hooks
    try:
        if "/root/.axon_site" not in sys.path:
            sys.path.append("/root/.axon_site")
        from trn_agent_boot import trn_boot
        so = "/opt/axon/libaxon_pjrt.so"
        if os.path.exists(so):
            hooks.set_axon_ntff_profile_hook(trn_boot._ntff_profile_via_ctypes(so))
    except Exception:
        pass


_install_ntff_hook()


def _ap(src, offset, dims):
    """Raw access pattern on the tensor behind AP/TensorHandle `src`.

    `offset` is relative to `src`'s own offset (elements)."""
    if isinstance(src, bass.AP):
        t, base = src.tensor, src.offset
    else:
        a = src[:]
        t, base = a.tensor, a.offset
    return bass.AP(tensor=t, offset=base + offset, ap=[list(d) for d in dims])


def build_nc():
    nc = bacc.Bacc("TRN2", target_bir_lowering=False, debug=False, num_devices=NCORES)

    def din(name, shape, dtype=F32):
        return nc.dram_tensor(name, list(shape), dtype, kind="ExternalInput")

    pair_loc = din("pair_loc", [B, LLOC, L, CZ])
    rots_loc = din("rots_loc", [B, LLOC, 9])
    trans_loc = din("trans_loc", [B, LLOC, 3])
    single_loc = din("single_loc", [B, LLOC, CS])
    t_in = din("t", [B])
    frame_w = din("frame_w", [12, C]); frame_b = din("frame_b", [1, C])
    single_w = din("single_w", [CS, C]); single_b = din("single_b", [1, C])
    tw1 = din("tw1", [C, 4 * C]); tb1 = din("tb1", [1, 4 * C])
    tw2 = din("tw2", [4 * C, C]); tb2 = din("tb2", [1, C])
    out_w = din("out_w", [C, 6]); out_b = din("out_b", [1, 6])
    ag1 = din("ag1", [NB, C]); abeta1 = din("abeta1", [NB, C])
    apw1 = din("apw1", [NB, C, 2 * C]); apb1 = din("apb1", [NB, 2 * C])
    ag2 = din("ag2", [NB, C]); abeta2 = din("abeta2", [NB, C])
    apw2 = din("apw2", [NB, C, 2 * C]); apb2 = din("apb2", [NB, 2 * C])
    wq = din("wq", [NB, C, C]); wk = din("wk", [NB, C, C])
    wv = din("wv", [NB, C, C]); wo = din("wo", [NB, C, C])
    wob = din("wob", [NB, C]); pw = din("pw", [NB, CZ, H])
    fw1 = din("fw1", [NB, C, 4 * C]); fb1 = din("fb1", [NB, 4 * C])
    fw2 = din("fw2", [NB, 4 * C, C]); fb2 = din("fb2", [NB, C])
    freqs = din("freqs", [1, C // 2])
    eye_f = din("eye_f", [128, 128])
    eye_b = din("eye_b", [128, 128], BF16)
    out_d = nc.dram_tensor("out", [B, LLOC, 12], F32, kind="ExternalOutput")

    with tile.TileContext(nc) as tc:
        import contextlib
        ctx = contextlib.ExitStack()
        with ctx:
            P = ctx.enter_context(tc.tile_pool(name="persist", bufs=1))
            work = ctx.enter_context(tc.tile_pool(name="work", bufs=2))
            bcpool = ctx.enter_context(tc.tile_pool(name="bcast", bufs=2))
            ps_t = ctx.enter_context(tc.tile_pool(name="ps_t", bufs=2, space="PSUM"))
            ps_s = ctx.enter_context(tc.tile_pool(name="ps_s", bufs=2, space="PSUM"))
            ps_a = ctx.enter_context(tc.tile_pool(name="ps_a", bufs=2, space="PSUM"))
            ps_m = ctx.enter_context(tc.tile_pool(name="ps_m", bufs=2, space="PSUM"))
            dram = ctx.enter_context(tc.tile_pool(name="dram", bufs=2, space="DRAM"))
            dramP = ctx.enter_context(tc.tile_pool(name="dramP", bufs=1, space="DRAM"))
            hpool = ctx.enter_context(tc.tile_pool(name="hpool", bufs=4))

            def psum(pool, shape, dtype=F32, tag=""):
                tg = tag or {id(ps_t): "t", id(ps_s): "s", id(ps_a): "a", id(ps_m): "m"}[id(pool)]
                return pool.tile(shape, dtype, tag=tg, name=f"ps{tg}_{nc.next_id()}")

            # ---------- constants ----------
            eyef_sb = P.tile([128, 128], F32)
            nc.sync.dma_start(out=eyef_sb, in_=eye_f[:])
            eyeb_sb = P.tile([128, 128], BF16)
            nc.sync.dma_start(out=eyeb_sb, in_=eye_b[:])
            ones_f = P.tile([1, 128], F32); nc.vector.memset(ones_f, 1.0)
            ones_b = P.tile([1, 128], BF16); nc.vector.memset(ones_b, 1.0)
            eps_ln = P.tile([128, 1], F32); nc.vector.memset(eps_ln, 1e-5)
            halfpi = P.tile([128, 1], F32); nc.vector.memset(halfpi, math.pi / 2)
            eps8 = P.tile([128, 1], F32); nc.vector.memset(eps8, 1e-8)
            one_c = P.tile([128, 1], F32); nc.vector.memset(one_c, 1.0)

            setup_ctx = contextlib.ExitStack()
            setup = setup_ctx.enter_context(tc.tile_pool(name="setup", bufs=1))

            # ---------- resident weights (bf16 via SWDGE cast-DMA) ----------
            def cast_w(src, blk, kc, n, name):
                tl = P.tile([128, kc, n], BF16, name=name)
                nc.gpsimd.dma_start(
                    out=tl, in_=_ap(src, blk * kc * 128 * n, [[n, 128], [128 * n, kc], [1, n]]))
                return tl

            wq_sb = [cast_w(wq, i, 2, C, f"wq{i}") for i in range(NB)]
            wk_sb = [cast_w(wk, i, 2, C, f"wk{i}") for i in range(NB)]
            wv_sb = [cast_w(wv, i, 2, C, f"wv{i}") for i in range(NB)]
            wo_sb = [cast_w(wo, i, 2, C, f"wo{i}") for i in range(NB)]
            fw1_sb = [cast_w(fw1, i, 2, 4 * C, f"fw1_{i}") for i in range(NB)]
            fw2_sb = [cast_w(fw2, i, 8, C, f"fw2_{i}") for i in range(NB)]

            pw_bd = P.tile([128, 2 * CH], BF16)
            nc.vector.memset(pw_bd, 0.0)
            for s in range(2):
                nc.gpsimd.dma_start(
                    out=pw_bd[s * CZ:(s + 1) * CZ, s * CH:s * CH + CH],
                    in_=_ap(pw, 0, [[H, CZ], [CZ * H, NB], [1, H]]))

            wob_sb = P.tile([1, NB * C], BF16)
            nc.gpsimd.dma_start(out=wob_sb, in_=_ap(wob, 0, [[NB * C, 1], [1, NB * C]]))
            fb2_sb = P.tile([1, NB * C], BF16)
            nc.gpsimd.dma_start(out=fb2_sb, in_=_ap(fb2, 0, [[NB * C, 1], [1, NB * C]]))

            # fb1 columns: [128, 8(hid-chunk), NB]
            fb1_sb = P.tile([128, 8, NB], F32)
            for k in range(8):
                fb1_nat = setup.tile([NB, 128], F32, tag="fb1n")
                nc.sync.dma_start(out=fb1_nat, in_=_ap(
                    fb1, k * 128, [[4 * C, NB], [1, 128]]))
                tps = psum(ps_t, [128, NB], F32)
                nc.tensor.transpose(tps, fb1_nat, eyef_sb[0:NB, 0:NB])
                nc.any.tensor_copy(out=fb1_sb[:, k, :], in_=tps)

            outw_sb = P.tile([128, 2, 6], F32)
            nc.sync.dma_start(out=outw_sb, in_=_ap(out_w, 0, [[6, 128], [768, 2], [1, 6]]))
            outb_sb = P.tile([1, 6], F32)
            nc.sync.dma_start(out=outb_sb, in_=out_b[:])

            frame_w_sb = setup.tile([12, C], F32)
            nc.sync.dma_start(out=frame_w_sb, in_=frame_w[:])
            single_w_sb = setup.tile([128, 2, C], F32)
            nc.sync.dma_start(out=single_w_sb, in_=_ap(single_w, 0, [[C, 128], [128 * C, 2], [1, C]]))
            cb_f = setup.tile([1, C], F32)
            cb_s = work.tile([1, C], F32)
            nc.sync.dma_start(out=cb_f, in_=frame_b[:])
            nc.sync.dma_start(out=cb_s, in_=single_b[:])
            nc.vector.tensor_add(out=cb_f, in0=cb_f, in1=cb_s)  # frame_b + single_b

            # ---------- h init ----------
            rots_sb, trans_sb, h_sb = [], [], []
            for b in range(B):
                rt = P.tile([LLOC, 9], F32, name=f"rots{b}")
                nc.sync.dma_start(out=rt, in_=rots_loc[b])
                tr = P.tile([LLOC, 3], F32, name=f"trans{b}")
                nc.sync.dma_start(out=tr, in_=trans_loc[b])
                rots_sb.append(rt); trans_sb.append(tr)

                ff = setup.tile([LLOC, 12], F32)
                nc.vector.tensor_copy(out=ff[:, 0:9], in_=rt)
                nc.vector.tensor_copy(out=ff[:, 9:12], in_=tr)
                ffT_ps = psum(ps_t, [12, LLOC], F32)
                nc.tensor.transpose(ffT_ps, ff, eyef_sb[0:LLOC, 0:LLOC])
                ffT = setup.tile([12, LLOC], F32)
                nc.any.tensor_copy(out=ffT, in_=ffT_ps)

                sg = setup.tile([LLOC, CS], F32)
                nc.sync.dma_start(out=sg, in_=single_loc[b])
                sgT = setup.tile([128, 2, LLOC], F32)
                for cc in range(2):
                    sps = psum(ps_t, [128, LLOC], F32)
                    nc.tensor.transpose(sps, sg[:, cc * 128:(cc + 1) * 128], eyef_sb[0:LLOC, 0:LLOC])
                    nc.any.tensor_copy(out=sgT[:, cc, :], in_=sps)

                hps = psum(ps_m, [LLOC, C], F32)
                nc.tensor.matmul(hps, ffT, frame_w_sb, start=True, stop=False)
                for cc in range(2):
                    nc.tensor.matmul(hps, sgT[:, cc, :], single_w_sb[:, cc, :],
                                     start=False, stop=False)
                nc.tensor.matmul(hps, ones_f[:, 0:LLOC], cb_f, start=False, stop=True)
                ht = hpool.tile([LLOC, C], F32, tag=f"h{b}", name=f"h_{b}")
                nc.vector.tensor_copy(out=ht, in_=hps)
                h_sb.append(ht)

            # ---------- time embedding -> adaLN row vectors ----------
            tb1_sb = setup.tile([1, 4 * C], F32)
            nc.sync.dma_start(out=tb1_sb, in_=tb1[:])
            tb2_sb = setup.tile([1, C], F32)
            nc.sync.dma_start(out=tb2_sb, in_=tb2[:])

            tsb = setup.tile([B, 1], F32)
            nc.sync.dma_start(out=tsb, in_=_ap(t_in, 0, [[1, B], [1, 1]]))
            fr2 = setup.tile([B, C // 2], F32)
            nc.sync.dma_start(out=fr2, in_=_ap(freqs, 0, [[0, B], [1, C // 2]]))
            targ = setup.tile([B, C // 2], F32)
            nc.vector.tensor_scalar_mul(out=targ, in0=fr2, scalar1=tsb)
            temb = setup.tile([B, C], F32)
            nc.scalar.activation(out=temb[:, 0:C // 2], in_=targ, func=AF.Sin,
                                 bias=halfpi[0:B], scale=1.0)
            nc.scalar.activation(out=temb[:, C // 2:C], in_=targ, func=AF.Sin)

            tembT = setup.tile([128, 2, B], F32)
            for cc in range(2):
                tps = psum(ps_t, [128, B], F32)
                nc.tensor.transpose(tps, temb[:, cc * 128:(cc + 1) * 128], eyef_sb[0:B, 0:B])
                nc.any.tensor_copy(out=tembT[:, cc, :], in_=tps)

            gT = setup.tile([128, 8, B], F32)
            for half in range(2):
                hd_ps = psum(ps_m, [B, 512], F32)
                for cc in range(2):
                    tw1_s = setup.tile([128, 512], F32, tag="tw1s")
                    nc.sync.dma_start(out=tw1_s, in_=_ap(
                        tw1, cc * 128 * 1024 + half * 512, [[1024, 128], [1, 512]]))
                    nc.tensor.matmul(hd_ps, tembT[:, cc, :], tw1_s,
                                     start=(cc == 0), stop=False)
                nc.tensor.matmul(hd_ps, ones_f[:, 0:B], tb1_sb[:, half * 512:(half + 1) * 512],
                                 start=False, stop=True)
                gmlp_h = setup.tile([B, 512], F32, tag="gmlph")
                nc.scalar.activation(out=gmlp_h, in_=hd_ps, func=AF.Gelu)
                for k4 in range(4):
                    tps = psum(ps_t, [128, B], F32)
                    nc.tensor.transpose(tps, gmlp_h[:, k4 * 128:(k4 + 1) * 128],
                                        eyef_sb[0:B, 0:B])
                    nc.any.tensor_copy(out=gT[:, half * 4 + k4, :], in_=tps)
            tc_ps = psum(ps_m, [B, C], F32)
            for k in range(8):
                tw2_s = setup.tile([128, C], F32, tag="tw2s")
                nc.sync.dma_start(out=tw2_s, in_=_ap(
                    tw2, k * 128 * C, [[C, 128], [1, C]]))
                nc.tensor.matmul(tc_ps, gT[:, k, :], tw2_s, start=(k == 0), stop=False)
            nc.tensor.matmul(tc_ps, ones_f[:, 0:B], tb2_sb, start=False, stop=True)
            tcond = setup.tile([B, C], F32)
            nc.vector.tensor_copy(out=tcond, in_=tc_ps)
            tcT = setup.tile([128, 2, B], F32)
            for cc in range(2):
                tps = psum(ps_t, [128, B], F32)
                nc.tensor.transpose(tps, tcond[:, cc * 128:(cc + 1) * 128], eyef_sb[0:B, 0:B])
                nc.any.tensor_copy(out=tcT[:, cc, :], in_=tps)

            # adaLN (m, s) row vectors for all (blk, which, b), staged in DRAM
            # so they can be partition-broadcast-loaded at block time.
            mrow_d = dramP.tile([NB * 2 * B, C], F32)
            srow_d = dramP.tile([NB * 2 * B, C], F32)
            apw_l = [apw1, apw2]; apb_l = [apb1, apb2]
            ag_l = [ag1, ag2]; ab_l = [abeta1, abeta2]
            for blk in range(NB):
                for wch in range(2):
                    apw_sb = setup.tile([128, 2, 2 * C], F32, tag="apw")
                    nc.sync.dma_start(out=apw_sb, in_=_ap(
                        apw_l[wch], blk * C * 2 * C, [[2 * C, 128], [128 * 2 * C, 2], [1, 2 * C]]))
                    apb_sb = setup.tile([1, 2 * C], F32, tag="apb")
                    nc.sync.dma_start(out=apb_sb, in_=_ap(apb_l[wch], blk * 2 * C, [[0, 1], [1, 2 * C]]))
                    ss_ps = psum(ps_m, [B, 2 * C], F32)
                    for cc in range(2):
                        nc.tensor.matmul(ss_ps, tcT[:, cc, :], apw_sb[:, cc, :],
                                         start=(cc == 0), stop=False)
                    nc.tensor.matmul(ss_ps, ones_f[:, 0:B], apb_sb, start=False, stop=True)
                    ag_bc = setup.tile([B, C], F32, tag="agbc")
                    nc.sync.dma_start(out=ag_bc, in_=_ap(ag_l[wch], blk * C, [[0, B], [1, C]]))
                    ab_bc = setup.tile([B, C], F32, tag="abbc")
                    nc.sync.dma_start(out=ab_bc, in_=_ap(ab_l[wch], blk * C, [[0, B], [1, C]]))
                    onep = setup.tile([B, C], F32, tag="onep")
                    nc.vector.tensor_scalar_add(out=onep, in0=ss_ps[:, 0:C], scalar1=1.0)
                    mr = setup.tile([B, C], F32, tag="mr")
                    nc.vector.tensor_mul(out=mr, in0=onep, in1=ag_bc)
                    sr = setup.tile([B, C], F32, tag="sr")
                    nc.vector.tensor_mul(out=sr, in0=onep, in1=ab_bc)
                    nc.vector.tensor_add(out=sr, in0=sr, in1=ss_ps[:, C:2 * C])
                    row = (blk * 2 + wch) * B
                    nc.sync.dma_start(out=mrow_d[row:row + B, :], in_=mr)
                    nc.sync.dma_start(out=srow_d[row:row + B, :], in_=sr)

            setup_ctx.close()

            # ---------- pair bias for all blocks ----------
            slabp = ctx.enter_context(tc.tile_pool(name="slab", bufs=3))
            ptp = ctx.enter_context(tc.tile_pool(name="pairT", bufs=4))
            escp = ctx.enter_context(tc.tile_pool(name="esc", bufs=8))
            bias_sb = P.tile([128, B * LLOC * NK * CH], BF16)  # [128, 36864]
            with nc.named_scope("pairproj"):
                for b in range(B):
                    for i0 in range(0, LLOC, IB):
                        slab = slabp.tile([128, IB, 384], BF16, tag="slab")
                        nc.gpsimd.dma_start(out=slab, in_=_ap(
                            pair_loc, (b * LLOC + i0) * L * CZ,
                            [[384, 128], [L * CZ, IB], [1, 384]]))
                        for ii in range(IB):
                            i = i0 + ii
                            pt_ps = psum(ps_t, [128, 3, 128], BF16)
                            for t3 in range(3):
                                nc.tensor.transpose(
                                    pt_ps[:, t3, :],
                                    slab[:, ii, t3 * 128:(t3 + 1) * 128], eyeb_sb)
                            ptsb = ptp.tile([128, 3, 128], BF16, tag="pt")
                            if i % 2 == 0:
                                nc.vector.tensor_copy(out=ptsb, in_=pt_ps)
                            else:
                                nc.scalar.copy(out=ptsb, in_=pt_ps)
                            bps = psum(ps_s, [128, 3, 2 * CH], F32)
                            for t3 in range(3):
                                nc.tensor.matmul(bps[:, t3, :], ptsb[:, t3, :], pw_bd,
                                                 start=True, stop=True)
                            off = (b * LLOC + i) * NK * CH
                            if i % 2 == 0:
                                nc.scalar.copy(out=bias_sb[:, off:off + NK * CH], in_=bps)
                            else:
                                nc.vector.tensor_copy(
                                    out=bias_sb[:, off:off + NK * CH], in_=bps)

            # ---------- transformer blocks ----------
            kT_sb = [P.tile([128, 2, L], BF16, name=f"kT{b}") for b in range(B)]
            vaug = [P.tile([128, NK, 33 * H], BF16, name=f"vaug{b}") for b in range(B)]
            for b in range(B):
                nc.vector.memset(vaug[b], 1.0)
            qT_sb = [P.tile([128, 2, LLOC], BF16, name=f"qT{b}") for b in range(B)]
            oT_sb = [P.tile([128, 2, LLOC], BF16, name=f"oT{b}") for b in range(B)]
            hhT_sb = [P.tile([128, 2, LLOC], BF16, name=f"hhT{b}") for b in range(B)]
            hhTf_sb = [P.tile([128, 2, L], BF16, name=f"hhTf{b}") for b in range(B)]
            h2T_sb = [P.tile([128, 2, LLOC], BF16, name=f"h2T{b}") for b in range(B)]

            def adaln(blk, wch, b, src):
                """adaLN of src [LLOC, C] f32 -> bf16 tile [LLOC, C]."""
                stats = work.tile([LLOC, 6], F32, tag="bnst")
                nc.vector.bn_stats(out=stats, in_=src)
                mv = work.tile([LLOC, 2], F32, tag="bnmv")
                nc.vector.bn_aggr(out=mv, in_=stats)
                nc.scalar.activation(out=mv[:, 1:2], in_=mv[:, 1:2], func=AF.Sqrt,
                                     bias=eps_ln[0:LLOC], scale=1.0)
                nc.vector.reciprocal(out=mv[:, 1:2], in_=mv[:, 1:2])
                xh = work.tile([LLOC, C], F32, tag="xh")
                nc.vector.tensor_scalar(out=xh, in0=src, scalar1=mv[:, 0:1],
                                        scalar2=mv[:, 1:2],
                                        op0=mybir.AluOpType.subtract,
                                        op1=mybir.AluOpType.mult)
                off = ((blk * 2 + wch) * B + b) * C
                m_bc = bcpool.tile([LLOC, C], F32, tag="mbc")
                nc.sync.dma_start(out=m_bc, in_=_ap(mrow_d, off, [[0, LLOC], [1, C]]))
                s_bc = bcpool.tile([LLOC, C], F32, tag="sbc")
                nc.sync.dma_start(out=s_bc, in_=_ap(srow_d, off, [[0, LLOC], [1, C]]))
                nc.vector.tensor_mul(out=xh, in0=xh, in1=m_bc)
                ob = work.tile([LLOC, C], BF16, tag="adaout")
                nc.vector.tensor_add(out=ob, in0=xh, in1=s_bc)
                return ob

            def transpose_to(dst, src_bf):
                """src_bf [LLOC, C] bf16 -> dst [128, 2, LLOC] bf16 (PE transpose)."""
                for cc in range(2):
                    tps = psum(ps_t, [128, LLOC], BF16)
                    nc.tensor.transpose(tps, src_bf[:, cc * 128:(cc + 1) * 128],
                                        eyeb_sb[0:LLOC, 0:LLOC])
                    nc.any.tensor_copy(out=dst[:, cc, :], in_=tps)

            # bias view with free dims ordered (channel, i) to match score tiles
            bias_r = bias_sb.rearrange("p (bb ii kk cc) -> p bb kk cc ii",
                                       bb=B, ii=LLOC, kk=NK, cc=CH)

            for blk in range(NB):
                with nc.named_scope(f"blk{blk}"):
                    cc_in = dram.tile([B, 128, 2, LLOC], BF16, tag="ccin")
                    for b in range(B):
                        hh = adaln(blk, 0, b, h_sb[b])
                        transpose_to(hhT_sb[b], hh)
                        nc.sync.dma_start(out=cc_in[b], in_=hhT_sb[b])
                        # local q while the collective runs
                        for dc in range(2):
                            qps = psum(ps_m, [128, LLOC], F32)
                            for cc in range(2):
                                nc.tensor.matmul(
                                    qps, wq_sb[blk][:, cc, dc * 128:(dc + 1) * 128],
                                    hhT_sb[b][:, cc, :], start=(cc == 0), stop=(cc == 1))
                            nc.scalar.activation(out=qT_sb[b][:, dc, :], in_=qps,
                                                 func=AF.Copy, scale=SCALE)

                    cc_out = dram.tile([NCORES, B, 128, 2, LLOC], BF16, tag="ccout")
                    nc.gpsimd.collective_compute(
                        "AllGather", mybir.AluOpType.bypass,
                        replica_groups=[list(range(NCORES))],
                        ins=[cc_in.opt()], outs=[cc_out.opt()])

                    for b in range(B):
                        for cc in range(2):
                            nc.sync.dma_start(out=hhTf_sb[b][:, cc, :], in_=_ap(
                                cc_out, b * (128 * 2 * LLOC) + cc * LLOC,
                                [[2 * LLOC, 128], [B * 128 * 2 * LLOC, NCORES], [1, LLOC]]))
                        # K^T: [d, j] tiles
                        for dc in range(2):
                            for half, n0, nn in ((0, 0, 512), (1, 512, 256)):
                                kps = psum(ps_m, [128, nn], F32, tag="m")
                                for cc in range(2):
                                    nc.tensor.matmul(
                                        kps, wk_sb[blk][:, cc, dc * 128:(dc + 1) * 128],
                                        hhTf_sb[b][:, cc, n0:n0 + nn],
                                        start=(cc == 0), stop=(cc == 1))
                                if (dc + half) % 2 == 0:
                                    nc.vector.tensor_copy(
                                        out=kT_sb[b][:, dc, n0:n0 + nn], in_=kps)
                                else:
                                    nc.scalar.copy(
                                        out=kT_sb[b][:, dc, n0:n0 + nn], in_=kps)
                        # V (permuted j order): [j, d] tiles + ones column
                        for kap in range(NK):
                            vps = psum(ps_m, [128, C], F32)
                            for cc in range(2):
                                lh = hhTf_sb[b][:, cc, :].rearrange(
                                    "p (n six) -> p six n", six=NK)[:, kap, :]
                                nc.tensor.matmul(vps, lh, wv_sb[blk][:, cc, :],
                                                 start=(cc == 0), stop=(cc == 1))
                            vdst = vaug[b].rearrange("p k (hh tt) -> p k hh tt",
                                                     hh=H)[:, kap, :, 0:HD]
                            vsrc = vps.rearrange("p (hh dd) -> p hh dd", hh=H)
                            if kap % 2 == 0:
                                nc.vector.tensor_copy(out=vdst, in_=vsrc)
                            else:
                                nc.scalar.copy(out=vdst, in_=vsrc)

                        # attention: scores transposed [j, (4 heads x i)] via a
                        # block-diagonal q tile (one matmul covers 4 heads);
                        # kappa-wide bias-add + exp; AV lands natural [i, d].
                        o_nat = work.tile([LLOC, C], BF16, tag="onat")
                        for dc in range(2):
                            q4 = work.tile([128, 4, LLOC], BF16, tag="q4")
                            nc.gpsimd.memset(q4, 0.0)
                            for hh in range(4):
                                nc.vector.tensor_copy(
                                    out=q4[hh * HD:(hh + 1) * HD, hh, :],
                                    in_=qT_sb[b][hh * HD:(hh + 1) * HD, dc, :])
                            escs = []
                            for kap in range(NK):
                                sps = psum(ps_s, [128, 4 * LLOC], F32)
                                lh = kT_sb[b][:, dc, :].rearrange(
                                    "p (n six) -> p six n", six=NK)[:, kap, :]
                                nc.tensor.matmul(sps, lh, q4.rearrange("p h i -> p (h i)"),
                                                 start=True, stop=True)
                                badd = work.tile([128, 4, LLOC], F32, tag="badd")
                                nc.vector.tensor_add(
                                    out=badd, in0=sps.rearrange("p (h i) -> p h i", h=4),
                                    in1=bias_r[:, b, kap,
                                               blk * H + dc * 4:blk * H + dc * 4 + 4, :])
                                esc = escp.tile([128, 4, LLOC], BF16, tag="esc")
                                nc.scalar.activation(out=esc, in_=badd, func=AF.Exp)
                                escs.append(esc)
                            for hh in range(4):
                                h = dc * 4 + hh
                                avps = psum(ps_a, [LLOC, 33], F32)
                                for kap in range(NK):
                                    nc.tensor.matmul(
                                        avps, escs[kap][:, hh, :],
                                        vaug[b][:, kap, h * 33:(h + 1) * 33],
                                        start=(kap == 0), stop=(kap == NK - 1))
                                rcp = work.tile([LLOC, 1], F32, tag="rcp")
                                nc.vector.reciprocal(out=rcp, in_=avps[:, 32:33])
                                nc.vector.tensor_scalar_mul(
                                    out=o_nat[:, h * HD:(h + 1) * HD],
                                    in0=avps[:, 0:HD], scalar1=rcp)
                        transpose_to(oT_sb[b], o_nat)

                        # h += o @ wo + wob
                        ups = psum(ps_m, [LLOC, C], F32)
                        for cc in range(2):
                            nc.tensor.matmul(ups, oT_sb[b][:, cc, :], wo_sb[blk][:, cc, :],
                                             start=(cc == 0), stop=False)
                        nc.tensor.matmul(ups, ones_b[:, 0:LLOC], wob_sb[:, blk * C:(blk + 1) * C],
                                         start=False, stop=True)
                        hmid = hpool.tile([LLOC, C], F32, tag=f"h{b}", name=f"hmid{blk}_{b}")
                        nc.vector.tensor_add(out=hmid, in0=h_sb[b], in1=ups)

                        # FFN
                        h2 = adaln(blk, 1, b, hmid)
                        transpose_to(h2T_sb[b], h2)
                        gT = work.tile([128, 8, LLOC], BF16, tag="gT")
                        for mc in range(8):
                            gps = psum(ps_m, [128, LLOC], F32)
                            for cc in range(2):
                                nc.tensor.matmul(
                                    gps, fw1_sb[blk][:, cc, mc * 128:(mc + 1) * 128],
                                    h2T_sb[b][:, cc, :], start=(cc == 0), stop=(cc == 1))
                            nc.scalar.activation(out=gT[:, mc, :], in_=gps, func=AF.Gelu,
                                                 bias=fb1_sb[:, mc, blk:blk + 1], scale=1.0)
                        fps = psum(ps_m, [LLOC, C], F32)
                        for mc in range(8):
                            nc.tensor.matmul(fps, gT[:, mc, :], fw2_sb[blk][:, mc, :],
                                             start=(mc == 0), stop=False)
                        nc.tensor.matmul(fps, ones_b[:, 0:LLOC], fb2_sb[:, blk * C:(blk + 1) * C],
                                         start=False, stop=True)
                        hnew = hpool.tile([LLOC, C], F32, tag=f"h{b}", name=f"hnew{blk}_{b}")
                        nc.vector.tensor_add(out=hnew, in0=hmid, in1=fps)
                        h_sb[b] = hnew

            # ---------- output head: corr -> rodrigues -> compose ----------
            with nc.named_scope("outhead"):
                for b in range(B):
                    hT = work.tile([128, 2, LLOC], F32, tag="hT")
                    for cc in range(2):
                        tps = psum(ps_t, [128, LLOC], F32)
                        nc.tensor.transpose(tps, h_sb[b][:, cc * 128:(cc + 1) * 128],
                                            eyef_sb[0:LLOC, 0:LLOC])
                        nc.any.tensor_copy(out=hT[:, cc, :], in_=tps)
                    cps = psum(ps_m, [LLOC, 6], F32)
                    for cc in range(2):
                        nc.tensor.matmul(cps, hT[:, cc, :], outw_sb[:, cc, :],
                                         start=(cc == 0), stop=False)
                    nc.tensor.matmul(cps, ones_f[:, 0:LLOC], outb_sb, start=False, stop=True)
                    corr = work.tile([LLOC, 6], F32, tag="corr")
                    nc.vector.tensor_copy(out=corr, in_=cps)

                    v3 = corr[:, 0:3]
                    vv = work.tile([LLOC, 3], F32, tag="vv")
                    nc.vector.tensor_mul(out=vv, in0=v3, in1=v3)
                    n2 = work.tile([LLOC, 1], F32, tag="n2")
                    nc.vector.reduce_sum(out=n2, in_=vv, axis=mybir.AxisListType.X)
                    nrm = work.tile([LLOC, 1], F32, tag="nrm")
                    nc.scalar.activation(out=nrm, in_=n2, func=AF.Sqrt)
                    sinn = work.tile([LLOC, 1], F32, tag="sinn")
                    nc.scalar.activation(out=sinn, in_=nrm, func=AF.Sin)
                    cosn = work.tile([LLOC, 1], F32, tag="cosn")
                    nc.scalar.activation(out=cosn, in_=nrm, func=AF.Sin,
                                         bias=halfpi[0:LLOC], scale=1.0)
                    rn = work.tile([LLOC, 1], F32, tag="rn")
                    nc.vector.tensor_scalar_add(out=rn, in0=nrm, scalar1=1e-8)
                    nc.vector.reciprocal(out=rn, in_=rn)
                    ax = work.tile([LLOC, 3], F32, tag="ax")
                    nc.vector.tensor_scalar_mul(out=ax, in0=v3, scalar1=rn)
                    sa = work.tile([LLOC, 3], F32, tag="sa")
                    nc.vector.tensor_scalar_mul(out=sa, in0=ax, scalar1=sinn)
                    omc = work.tile([LLOC, 1], F32, tag="omc")
                    nc.vector.tensor_scalar(out=omc, in0=cosn, scalar1=-1.0,
                                            scalar2=1.0,
                                            op0=mybir.AluOpType.mult,
                                            op1=mybir.AluOpType.add)
                    R = work.tile([LLOC, 9], F32, tag="R")
                    for r in range(3):
                        nc.vector.tensor_scalar_mul(out=R[:, 3 * r:3 * r + 3], in0=ax,
                                                    scalar1=ax[:, r:r + 1])
                    nc.vector.tensor_scalar_mul(out=R, in0=R, scalar1=omc)
                    diag = _ap(R, 0, [list(R.ap[0]), [4, 3]])
                    nc.vector.tensor_scalar_add(out=diag, in0=diag, scalar1=cosn)
                    for col, src, sgn in ((1, 2, -1), (2, 1, +1), (3, 2, +1),
                                          (5, 0, -1), (6, 1, -1), (7, 0, +1)):
                        fn = nc.vector.tensor_add if sgn > 0 else nc.vector.tensor_sub
                        fn(out=R[:, col:col + 1], in0=R[:, col:col + 1],
                           in1=sa[:, src:src + 1])

                    res = work.tile([LLOC, 12], F32, tag="res")
                    tmp3 = work.tile([LLOC, 3], F32, tag="tmp3")
                    for r in range(3):
                        dst = res[:, 3 * r:3 * r + 3]
                        nc.vector.tensor_scalar_mul(out=dst, in0=R[:, 0:3],
                                                    scalar1=rots_sb[b][:, 3 * r:3 * r + 1])
                        for k in (1, 2):
                            nc.vector.tensor_scalar_mul(
                                out=tmp3, in0=R[:, 3 * k:3 * k + 3],
                                scalar1=rots_sb[b][:, 3 * r + k:3 * r + k + 1])
                            nc.vector.tensor_add(out=dst, in0=dst, in1=tmp3)
                    # new_trans = rots @ t_upd + trans
                    tup = corr[:, 3:6]
                    t1 = work.tile([LLOC, 3], F32, tag="t1")
                    t2 = work.tile([LLOC, 3], F32, tag="t2")
                    # rows of rots: res_t[r] = sum_k rots[3r+k]*tup[k]
                    rots_rk = rots_sb[b].rearrange("p (r k) -> p r k", k=3)
                    nc.vector.tensor_scalar_mul(out=t1, in0=rots_rk[:, :, 0],
                                                scalar1=tup[:, 0:1])
                    for k in (1, 2):
                        nc.vector.tensor_scalar_mul(out=t2, in0=rots_rk[:, :, k],
                                                    scalar1=tup[:, k:k + 1])
                        nc.vector.tensor_add(out=t1, in0=t1, in1=t2)
                    nc.vector.tensor_add(out=res[:, 9:12], in0=t1, in1=trans_sb[b])
                    nc.sync.dma_start(out=out_d[b], in_=res)

    nc.compile()
    return nc


def _inputs_to_maps(inputs):
    ins = {k: np.ascontiguousarray(np.asarray(v, dtype=np.float32)) for k, v in inputs.items()}
    half = C // 2
    freqs = np.exp(-math.log(10000.0) * np.arange(half, dtype=np.float32) / half)
    common = {
        "t": ins["t"],
        "frame_w": ins["frame_w"], "frame_b": ins["frame_b"].reshape(1, C),
        "single_w": ins["single_w"], "single_b": ins["single_b"].reshape(1, C),
        "tw1": ins["tw1"], "tb1": ins["tb1"].reshape(1, 4 * C),
        "tw2": ins["tw2"], "tb2": ins["tb2"].reshape(1, C),
        "out_w": ins["out_w"], "out_b": ins["out_b"].reshape(1, 6),
        "ag1": ins["ag1"], "abeta1": ins["abeta1"],
        "apw1": ins["apw1"], "apb1": ins["apb1"],
        "ag2": ins["ag2"], "abeta2": ins["abeta2"],
        "apw2": ins["apw2"], "apb2": ins["apb2"],
        "wq": ins["wq"], "wk": ins["wk"], "wv": ins["wv"], "wo": ins["wo"],
        "wob": ins["wob"], "pw": ins["pw"],
        "fw1": ins["fw1"], "fb1": ins["fb1"], "fw2": ins["fw2"], "fb2": ins["fb2"],
        "freqs": freqs.reshape(1, half),
        "eye_f": np.eye(128, dtype=np.float32),
        "eye_b": np.eye(128).astype(ml_dtypes.bfloat16),
    }
    maps = []
    rots9 = ins["rots"].reshape(B, L, 9)
    for c in range(NCORES):
        sl = slice(c * LLOC, (c + 1) * LLOC)
        m = dict(common)
        m["pair_loc"] = np.ascontiguousarray(ins["pair"][:, sl])
        m["rots_loc"] = np.ascontiguousarray(rots9[:, sl])
        m["trans_loc"] = np.ascontiguousarray(ins["trans"][:, sl])
        m["single_loc"] = np.ascontiguousarray(ins["single"][:, sl])
        maps.append(m)
    return maps


def kernel(**inputs):
    if "nc" not in _CACHED:
        _CACHED["nc"] = build_nc()
    nc = _CACHED["nc"]
    maps = _inputs_to_maps(inputs)
    res = run_bass_kernel_spmd(nc, maps, core_ids=list(range(NCORES)))
    _LAST["exec_time_ns"] = res.exec_time_ns
    _LAST["results"] = res
    out = np.concatenate([res.results[c]["out"] for c in range(NCORES)], axis=1)
    return out.astype(np.float32)
